# revision 17
# baseline (speedup 1.0000x reference)
"""VQ codebook reconstruction kernel for Trainium2 (8 NeuronCores, SPMD).

Reference computation (per pixel feature vector f in R^C):
    weights = (codebook @ f) / ||codebook_rows||^2      # [N]
    recon   = codebook.T @ weights                      # [C]

This collapses to a single fixed matrix applied per pixel:
    recon = M @ f,   M = codebook.T @ diag(1/||c_n||^2) @ codebook   # [C, C]

M is tiny ([256,256], symmetric), formed on the host in float64. The device
kernel applies M to all B*H*W = 131072 pixel vectors, sharded data-parallel
over (B, H-halves) across 8 cores.

v5 design (85us fp32r baseline -> 65us -> 62us -> this):
  - Whole data path fp16 (host pre-quantizes): 16.8 MB/core total HBM
    traffic at the ~420 GB/s per-core DMA ceiling -> ~40us stream floor.
    (fp8 input was simulated on host: max-rel err 3.8e-2 > the 2e-2 gate,
    so fp16 is the real floor.) fp16 streams the PE at 1 row/cycle and
    enables Fast Weight Load; PE work is ~29us, inside the DMA wall.
  - Variable slab sizes (1K first, 2K middle, 1K tail): fast first matmul,
    short final write.
  - Each slab's input is split by K-half across the two input queues
    (sync kb0 / gpsimd kb1); queue FIFO order == compute order.
  - Outputs split by M-half: scalar casts mb1 and issues its own output
    DMA (no cross-engine wait); vector casts mb0, gpsimd issues that DMA.
    Input issues run 2 slabs ahead of any output issue on the same engine.
  - kb-outer matmul order (mb-outer measured worse: PSUM contention).
  - PE warm-up matmuls on a memset tile flip the HAM clock gate to
    2.4 GHz during the input prefill; measured warm throughout after.
"""

import numpy as np

B, C, H, W = 4, 256, 128, 256
N_CORES = 8
SPLIT_H = 2            # 8 shards = B(4) x H-halves(2)
SH = H // SPLIT_H      # 64 rows of H per shard
P_SHARD = SH * W       # 16384 pixels per core
TILE_N = 512
SLABS = [512, 1024, 2048, 2048, 2048, 2048, 2048, 2048, 1024, 1024, 512]
assert sum(SLABS) == P_SHARD
OFFS = [sum(SLABS[:j]) for j in range(len(SLABS))]

_NC_CACHE = {}


def _build_nc():
    if "nc" in _NC_CACHE:
        return _NC_CACHE["nc"]

    import concourse.bass as bass
    import concourse.tile as tile
    from concourse import bacc, mybir

    f32 = mybir.dt.float32
    f16 = mybir.dt.float16

    nc = bacc.Bacc()
    # feat[p, a, q] = f[a*128+p, q]  (host pre-shuffled fp16)
    feat = nc.dram_tensor("feat", [128, 2, P_SHARD], f16, kind="ExternalInput")
    # mmat[p, a, c] = M[a*128+p, c]
    mmat = nc.dram_tensor("mmat", [128, 2, C], f16, kind="ExternalInput")
    # out[p, mb, q] = recon[mb*128+p, q]
    out = nc.dram_tensor("out", [128, 2, P_SHARD], f16, kind="ExternalOutput")

    with tile.TileContext(nc) as tc:
        with (
            tc.tile_pool(name="mpool", bufs=1) as mpool,
            tc.tile_pool(name="warm", bufs=1) as warm_pool,
            tc.tile_pool(name="rhs", bufs=5) as rhs_pool,
            tc.tile_pool(name="ov", bufs=4) as ov_pool,
            tc.tile_pool(name="os", bufs=4) as os_pool,
            tc.tile_pool(name="psum", bufs=2, space="PSUM") as psum_pool,
        ):
            mt = mpool.tile([128, 2, C], f16, tag="m")
            nc.sync.dma_start(mt[:], mmat[:, :, :])

            rts = [rhs_pool.tile([128, 2, sz], f16, tag="r", name=f"rt{j}")
                   for j, sz in enumerate(SLABS)]

            def issue_in(j):
                # K-half a rides queue a: service order == consume order.
                o, sz = OFFS[j], SLABS[j]
                nc.sync.dma_start(rts[j][:, 0, :], feat[:, 0, o:o + sz])
                nc.gpsimd.dma_start(rts[j][:, 1, :], feat[:, 1, o:o + sz])

            issue_in(0)
            issue_in(1)

            # PE warm-up: self-contained matmuls on a memset tile keep the
            # PE busy through the HAM activity window during input prefill.
            wt = warm_pool.tile([128, TILE_N], f16, tag="w")
            nc.vector.memset(wt[:], 1.0)
            for i in range(8):
                pw = psum_pool.tile([128, TILE_N], f32, tag="ps00", name=f"pw{i}")
                nc.tensor.matmul(pw[:], wt[:, 0:128], wt[:], start=True, stop=True)

            for j, sz in enumerate(SLABS):
                if j + 2 < len(SLABS):
                    issue_in(j + 2)
                o = OFFS[j]
                rt = rts[j]
                n_sub = sz // TILE_N
                otv = ov_pool.tile([128, sz], f16, tag="ov", name=f"otv{j}")
                ots = os_pool.tile([128, sz], f16, tag="os", name=f"ots{j}")
                for g in range(0, n_sub, 2):
                    w = min(2, n_sub - g)
                    ps = {}
                    for mb in range(2):
                        for n in range(w):
                            ps[(mb, n)] = psum_pool.tile(
                                [128, TILE_N], f32, tag=f"ps{mb}{n}", name=f"ps{mb}{n}"
                            )
                    # kb-outer: 4 weight switches per group, FWL-hidden.
                    for kb in range(2):
                        for mb in range(2):
                            for n in range(w):
                                nc.tensor.matmul(
                                    ps[(mb, n)][:],
                                    mt[:, kb, mb * 128:(mb + 1) * 128],
                                    rt[:, kb, bass.ts(g + n, TILE_N)],
                                    start=(kb == 0),
                                    stop=(kb == 1),
                                )
                    for n in range(w):
                        nc.vector.tensor_copy(
                            otv[:, bass.ts(g + n, TILE_N)], ps[(0, n)][:]
                        )
                        nc.scalar.copy(
                            ots[:, bass.ts(g + n, TILE_N)], ps[(1, n)][:]
                        )
                # mb0 output rides gpsimd; the last slabs ride sync instead
                # (sync's inputs are done by then) so 3 rings drain the tail.
                eng = nc.sync if j >= len(SLABS) - 2 else nc.gpsimd
                eng.dma_start(out[:, 0, o:o + sz], otv[:])
                nc.scalar.dma_start(out[:, 1, o:o + sz], ots[:])

    nc.compile()
    _NC_CACHE["nc"] = nc
    return nc


def _host_prep(feature, codebook):
    cb = codebook.astype(np.float64)
    norm = np.sum(cb * cb, axis=1)
    m = ((cb / norm[:, None]).T @ cb).astype(np.float32)
    # m3[p, a, c] = M[a*128+p, c]
    m3 = np.ascontiguousarray(
        m.reshape(2, 128, C).transpose(1, 0, 2).astype(np.float16)
    )

    in_maps = []
    for i in range(N_CORES):
        b, hs = i // SPLIT_H, (i % SPLIT_H) * SH
        shard = feature[b, :, hs:hs + SH, :].reshape(C, P_SHARD)
        # f3[p, a, q] = shard[a*128+p, q]
        f3 = np.ascontiguousarray(
            shard.reshape(2, 128, P_SHARD).transpose(1, 0, 2).astype(np.float16)
        )
        in_maps.append({"feat": f3, "mmat": m3})
    return in_maps


def _gather(results):
    out = np.empty((B, C, H, W), dtype=np.float32)
    for i in range(N_CORES):
        b, hs = i // SPLIT_H, (i % SPLIT_H) * SH
        o = results[i]["out"].astype(np.float32)   # [128, 2, P_SHARD]
        shard = o.transpose(1, 0, 2).reshape(C, SH, W)
        out[b, :, hs:hs + SH, :] = shard
    return out


def run(feature, codebook, **spmd_kwargs):
    from concourse.bass_utils import run_bass_kernel_spmd

    nc = _build_nc()
    in_maps = _host_prep(np.asarray(feature), np.asarray(codebook))
    res = run_bass_kernel_spmd(nc, in_maps, list(range(N_CORES)), **spmd_kwargs)
    return _gather(res.results), res


def kernel(feature, codebook):
    out, _ = run(feature, codebook)
    return out
